# revision 1
# baseline (speedup 1.0000x reference)
"""Trainium2 Bass kernel for nn_MinibatchDiscrimination.

Reference computation (N=256, A=1024, B=128, C=32):
    M  = einsum('na,abc->nbc', x, T)                      # (N,B,C)
    l1 = sum_c |M[n,b,c] - M[m,b,c]|                      # (N,N,B)
    o  = sum_m exp(-l1)                                   # (N,B)
    out = concat([x, o], axis=1)                          # (N, A+B)

Sharding: B (kernel dim) split across 8 cores, 16 kernels each. Each core
computes M^T for its 16 kernels (PE matmul), then for every kernel b the
pairwise-L1 / exp / batch-sum, emitting its (256,16) slice of o. The host
gathers the slices and concatenates with x.

The pairwise L1 uses the relu + rank-1 identity (the DVE tensor_scalar ISA
has no abs op, but (subtract, max 0) is valid):
    sum_c |d_c| = 2*sum_c relu(d_c) - S[m] + S[n],  d = M[m,:] - M[n,:],
    S[m] = sum_c M[m,c].
Both rank-1 terms are folded into the same PSUM accumulation via two K=1
matmuls using the same bf16-rounded S values, so the diagonal cancels to
exactly 0 and exp(0)=1 dominates o with full fp32 accuracy.

Per-core device pipeline:
  phase 1: MT[(b c), n] = Tl.T @ xT on PE (psum f32) -> sbuf bf16 (mb) and
           f32-of-the-same-bf16-values (mf), both resident.
  phase 2 (per b):
    R (128,256) bf16 = M_b^T replicated 4x along partition groups
    Bias (128,64) f32, column q = M[4q+g, b,:] per partition group g
    (both via partition-shifted gpsimd copies; NBias = -Bias for ACT quads)
    S row (1,256) via tiny PE matmul; negated/rounded to bf16 rows.
    per quad q (samples n=4q+g): DVE tensor_scalar(sub, max 0) emits
    relu(M[m,c]-M[n,c]) in bf16 (4x mode) — some quads on ACT (Relu+bias);
    PE reduces the 32-channel partition groups with a block-diagonal 2.0
    matmul into a 32-row PSUM strip (col-tiled, 8 quads accumulate per
    strip, strips interleaved for subarray concurrency).
    per 128-row block: two K=1 rank-1 matmuls add -S[m] (free dim) and
    +S[n] (partition dim); ACT computes exp(-D) with a fused free-dim
    accumulate producing o[n] directly.
"""

from contextlib import ExitStack

import numpy as np
import ml_dtypes

import concourse.bass as bass
import concourse.bacc as bacc
import concourse.tile as tile
from concourse import mybir
from concourse.bass_utils import run_bass_kernel_spmd

N, A, B, C = 256, 1024, 128, 32
NCORES = 8
BLOC = B // NCORES            # 16 kernels per core
BC = BLOC * C                 # 512 = (b,c) pairs per core
KT = A // 128                 # 8 contraction tiles
NQ = 64                       # quads per kernel b (4 samples each)

F32 = mybir.dt.float32
BF16 = mybir.dt.bfloat16
ALU = mybir.AluOpType
ACTF = mybir.ActivationFunctionType

_bf = ml_dtypes.bfloat16

# engine schedule for the relu quad-pairs, tuned from trace rates
# (DVE ~234ns/quad, ACT ~507ns/quad; GPSIMD is 17x slower - copies only)
QUAD_CYCLE = ["v", "v", "a"]


def _build_twos8() -> np.ndarray:
    """lhsT weight bank: 8 variants of (128,32) block-diagonal 2.0.

    Variant j (columns 32j..32j+32) has 2.0 at [g*32+c, 4j+g]: a matmul with
    rhs=relu tile sums each 32-channel partition group g (doubled) into
    strip row 4j+g."""
    w = np.zeros((128, 256), np.float32)
    for j in range(8):
        for g in range(4):
            w[g * 32:(g + 1) * 32, 32 * j + 4 * j + g] = 2.0
    return w.astype(_bf)


def build_nc():
    nc = bacc.Bacc("TRN2", target_bir_lowering=False, debug=False)

    xT_d = nc.declare_dram_parameter("xT", [A, N], BF16, isOutput=False)
    Tl_d = nc.declare_dram_parameter("Tl", [A, BC], BF16, isOutput=False)
    twos_d = nc.declare_dram_parameter("twos8", [128, 256], BF16, isOutput=False)
    onecol_d = nc.declare_dram_parameter("onecol", [128, 1], BF16, isOutput=False)
    onerow_d = nc.declare_dram_parameter("onerow", [1, N], BF16, isOutput=False)
    o_d = nc.declare_dram_parameter("o_raw", [2, 128, BLOC], F32, isOutput=True)

    xT = xT_d.ap()
    Tl = Tl_d.ap()
    o_out = o_d.ap()

    with tile.TileContext(nc) as tc, ExitStack() as ctx:
        singles = ctx.enter_context(tc.tile_pool(name="singles", bufs=1))

        twos_sb = singles.tile([128, 256], BF16, tag="twos8")
        nc.sync.dma_start(out=twos_sb[:], in_=twos_d.ap()[:, :])
        onecol_sb = singles.tile([128, 1], BF16, tag="onecol")
        nc.sync.dma_start(out=onecol_sb[:], in_=onecol_d.ap()[:, :])
        onerow_sb = singles.tile([1, N], BF16, tag="onerow")
        nc.sync.dma_start(out=onerow_sb[:], in_=onerow_d.ap()[:, :])

        xT_sb = []
        Tl_sb = []
        for k in range(KT):
            xk = singles.tile([128, N], BF16, tag=f"xT{k}")
            nc.sync.dma_start(out=xk[:], in_=xT[k * 128:(k + 1) * 128, :])
            xT_sb.append(xk)
            tk = singles.tile([128, BC], BF16, tag=f"Tl{k}")
            nc.sync.dma_start(out=tk[:], in_=Tl[k * 128:(k + 1) * 128, :])
            Tl_sb.append(tk)

        mb_sb = []   # bf16 M^T tiles, resident in SBUF
        mf_sb = []   # f32 M^T tiles (same bf16-rounded values), resident

        # ---- phase 1: MT[(b c), n] = sum_a Tl[a, bc] * xT[a, n] ----
        mtps = ctx.enter_context(tc.tile_pool(name="mtps", bufs=2, space="PSUM"))
        for jj in range(BC // 128):
            ps = mtps.tile([128, N], F32, tag="mt")
            for k in range(KT):
                nc.tensor.matmul(
                    ps[:],
                    Tl_sb[k][:, jj * 128:(jj + 1) * 128],
                    xT_sb[k][:],
                    start=(k == 0),
                    stop=(k == KT - 1),
                )
            mb = singles.tile([128, N], BF16, tag=f"mtbf{jj}")
            nc.vector.tensor_copy(mb[:], ps[:])
            mb_sb.append(mb)
            mf = singles.tile([128, N], F32, tag=f"mtf32{jj}")
            nc.scalar.copy(mf[:], mb[:])
            mf_sb.append(mf)

        # ---- phase 2 ----
        o_sb = singles.tile([128, 2 * BLOC], F32, tag="osb")

        rpool = ctx.enter_context(tc.tile_pool(name="rpool", bufs=3))
        biasp = ctx.enter_context(tc.tile_pool(name="biasp", bufs=3))
        nbias = ctx.enter_context(tc.tile_pool(name="nbias", bufs=3))
        abspool = ctx.enter_context(tc.tile_pool(name="abspool", bufs=8))
        srowp = ctx.enter_context(tc.tile_pool(name="srowp", bufs=3))
        edump = ctx.enter_context(tc.tile_pool(name="edump", bufs=2))
        dpool = ctx.enter_context(tc.tile_pool(name="dpool", bufs=3, space="PSUM"))
        auxps = ctx.enter_context(tc.tile_pool(name="auxps", bufs=2, space="PSUM"))

        qctr = 0
        for b in range(BLOC):
            jj, prow = b // 4, (b % 4) * 32
            R = rpool.tile([128, N], BF16, tag="R")
            Bias = biasp.tile([128, NQ], F32, tag="Bias")
            for g in range(4):
                nc.vector.tensor_copy(
                    R[g * 32:(g + 1) * 32, :],
                    mb_sb[jj][prow:prow + 32, :])
                # Bias[g*32+c, q] = MT[b*32+c, 4q+g]
                src = mf_sb[jj][prow:prow + 32, :].rearrange(
                    "c (q g) -> c g q", g=4)[:, g, :]
                nc.vector.tensor_copy(Bias[g * 32:(g + 1) * 32, :], src)
            NBias = nbias.tile([128, NQ], F32, tag="NBias")
            nc.vector.tensor_scalar_mul(NBias[:], Bias[:], -1.0)

            # S row: S[m] = sum_c M[m, b*32+c], via 32-partition ones matmul
            srow_ps = auxps.tile([1, N], F32, tag="srow")
            nc.tensor.matmul(
                srow_ps[:],
                onecol_sb[prow:prow + 32, 0:1],
                mb_sb[jj][prow:prow + 32, :],
                start=True, stop=True,
                tile_position=(prow, 0))
            # bf16-rounded +S and -S rows (shared by both rank-1 updates)
            posS = srowp.tile([1, N], BF16, tag="posS")
            nc.vector.tensor_copy(posS[:], srow_ps[:])
            negS = srowp.tile([1, N], BF16, tag="negS")
            nc.vector.tensor_scalar_mul(negS[:], srow_ps[:], -1.0)

            # D covers both 128-row blocks: cols [0:256) beta=0, [256:512) b=1
            D = dpool.tile([128, 2 * N], F32, tag="D")
            for i in range(32):
                s, j = i % 4, i // 4
                qh = 8 * s + j              # quad-in-block; rows 4qh+g
                rt = abspool.tile([128, 2 * N], BF16, tag="rt")
                for beta in range(2):
                    q = beta * 32 + qh
                    half = rt[:, beta * N:(beta + 1) * N]
                    eng = QUAD_CYCLE[qctr % len(QUAD_CYCLE)]
                    qctr += 1
                    if eng == "a":
                        nc.scalar.activation(
                            out=half, in_=R[:], func=ACTF.Relu,
                            bias=NBias[:, q:q + 1], scale=1.0)
                    elif eng == "g":
                        nc.gpsimd.tensor_scalar(
                            half, R[:], Bias[:, q:q + 1], 0.0,
                            ALU.subtract, ALU.max)
                    else:
                        nc.vector.tensor_scalar(
                            half, R[:], Bias[:, q:q + 1], 0.0,
                            ALU.subtract, ALU.max)
                nc.tensor.matmul(
                    D[32 * s:32 * s + 32, :],
                    twos_sb[:, 32 * j:32 * j + 32],
                    rt[:],
                    start=(j == 0),
                    stop=False,
                    tile_position=(0, 32 * s),
                    skip_group_check=True,
                )
            # rank-1 corrections: D += -S[m] (free) + S[n] (partition)
            negS2 = negS[0:1, :].unsqueeze(1).broadcast_to([1, 2, N])
            nc.tensor.matmul(
                D[:], onerow_sb[:, 0:128], negS2,
                start=False, stop=False, skip_group_check=True)
            for beta in range(2):
                nc.tensor.matmul(
                    D[:, beta * N:(beta + 1) * N],
                    posS[:, beta * 128:(beta + 1) * 128], onerow_sb[:],
                    start=False, stop=(beta == 1), skip_group_check=True)
            for beta in range(2):
                ed = edump.tile([128, N], BF16, tag="ed")
                nc.scalar.activation(
                    out=ed[:], in_=D[:, beta * N:(beta + 1) * N],
                    func=ACTF.Exp, scale=-1.0,
                    accum_out=o_sb[:, beta * BLOC + b:beta * BLOC + b + 1])

        for beta in range(2):
            nc.sync.dma_start(
                out=o_out[beta],
                in_=o_sb[:, beta * BLOC:(beta + 1) * BLOC])

    nc.compile()
    return nc


_NC = None


def _get_nc():
    global _NC
    if _NC is None:
        _NC = build_nc()
    return _NC


def _prep_inputs(x: np.ndarray, T: np.ndarray):
    xT_bf = np.ascontiguousarray(x.T).astype(_bf)
    twos8 = _build_twos8()
    onecol = np.ones((128, 1), np.float32).astype(_bf)
    onerow = np.ones((1, N), np.float32).astype(_bf)
    in_maps = []
    for core in range(NCORES):
        Tl = np.ascontiguousarray(
            T[:, core * BLOC:(core + 1) * BLOC, :].reshape(A, BC)).astype(_bf)
        in_maps.append({"xT": xT_bf, "Tl": Tl, "twos8": twos8,
                        "onecol": onecol, "onerow": onerow})
    return in_maps


def _assemble(x: np.ndarray, results) -> np.ndarray:
    o = np.zeros((N, B), np.float32)
    for core in range(NCORES):
        o_raw = results[core]["o_raw"]          # (2, 128, BLOC) f32
        o[:128, core * BLOC:(core + 1) * BLOC] = o_raw[0]
        o[128:, core * BLOC:(core + 1) * BLOC] = o_raw[1]
    return np.concatenate([x.astype(np.float32), o], axis=1)


def run_device(x: np.ndarray, T: np.ndarray, trace: bool = False):
    """Run the SPMD kernel; returns (full output, BassKernelResults)."""
    nc = _get_nc()
    in_maps = _prep_inputs(x, T)
    res = run_bass_kernel_spmd(nc, in_maps, list(range(NCORES)), trace=trace)
    return _assemble(x, res.results), res


def kernel(x: np.ndarray, T: np.ndarray) -> np.ndarray:
    x = np.asarray(x, dtype=np.float32)
    T = np.asarray(T, dtype=np.float32)
    out, _ = run_device(x, T)
    return out


if __name__ == "__main__":
    rng = np.random.default_rng(0)
    x = rng.standard_normal((N, A)).astype(np.float32)
    T = (rng.standard_normal((A, B, C)) * 0.05).astype(np.float32)
    out = kernel(x, T)
    print("out", out.shape, out.dtype)

